# revision 10
# baseline (speedup 1.0000x reference)
"""Concordance-index loss on Trainium2 (8 NeuronCores, Bass/Tile).

Reference math over N=8192 samples (t = exp(event_time), d = event_indicator,
r = estimate), pairwise over ordered pairs (i, j):
    comp(i,j)  = d_i & (t_i < t_j | (t_i == t_j & ~d_j))
    conc       = sum comp & (r_j - r_i < 0)
    tied       = sum comp & |r_j - r_i| <= 1e-8
    total      = sum comp
    disc       = total - conc - tied
    out        = 1 - (disc + 0.5*tied) / (disc + conc + tied + 1e-7)

Device strategy (host does only O(N log N) re-encoding of the three length-N
vectors; all 67M pairwise compares run on the NeuronCores):

 - t is quantized (0.05 grid in log space, and exp is strictly monotone), so
   replace t by its dense rank K_t < 2048.  The predicate
   (t_i < t_j | (t_i == t_j & ~d_j)) collapses to ONE compare
   trank_i < trank_j + 0.5*(1-d_j), and the d_i gate folds in by setting
   censored rows' trank_i to a +32768 sentinel.  All values are fp16-exact.
 - r is replaced by its dense rank, embedded as monotonically increasing fp16
   bit patterns (rank+1024 viewed as fp16) so order compares are exact and the
   DVE can run in 16-bit perf modes.  conc(i,j) = (remb_i > remb_j).
 - The tie band |fl(r_j - r_i)| <= 1e-8f is, for each j, a CONTIGUOUS window
   [lo_j, hi_j] of r-ranks (differences are monotone in the sorted order).
   Windows are found on host with the exact same IEEE f32 subtract the
   reference uses.  tied(i,j) = (remb_i <= hiemb_j) - (remb_i < loemb_j).

Sharding: each of the 8 cores owns a 1024-wide i-slice (free dim) and loops
all 8192 j as 64 partition-chunks of 128.  Per chunk, 4 VectorE instructions
(one tensor_scalar + three scalar_tensor_tensor) compute the masked compares
with inline accum_out row-sums; the host all-reduces the four count tiles and
applies the final scalar formula.
"""

import numpy as np

N = 8192
NCORES = 8
P = 128
IBLK = N // NCORES          # 1024 i's per core (free dim)
CCH = N // P                # 64 j partition-chunks

_CACHE = {}


def _build_nc():
    import concourse.bass as bass
    import concourse.tile as tile
    from concourse import mybir

    dt = mybir.dt
    Alu = mybir.AluOpType

    nc = bass.Bass()
    # All inputs byte-packed into ONE dram tensor (fp16 tm|re broadcast rows,
    # then f32 uj|rj|hij|loj scalars) so the whole kernel uses exactly two
    # DMAs (1 in + 1 out).  More DMA queues -> more semaphores -> the
    # kernel-tail Drain instruction exceeds its tiny ISA sync-wait budget.
    NB16 = 2 * IBLK * 2                 # bytes of fp16 payload per partition
    NB32 = 4 * CCH * 4                  # bytes of f32 payload per partition
    xin = nc.declare_dram_parameter("xin", [P, NB16 + NB32], dt.uint8,
                                    isOutput=False)
    out = nc.declare_dram_parameter("out", [P, 4 * CCH], dt.float32,
                                    isOutput=True)

    # Raw bass (no TileContext): the kernel is straight-line single-engine
    # DVE code, so program order gives all intra-engine dependencies; the
    # only syncs needed are DMA-in -> DVE and DVE -> DMA-out.  (TileContext's
    # tail Drain needs one sync-wait per proc, which exceeds the 2-slot ISA
    # budget of the Drain instruction once a DMA queue is involved.)
    with (
        nc.sbuf_tensor([P, NB16 + NB32], dt.uint8) as xin_s,
        nc.sbuf_tensor([P, 4 * CCH], dt.float32) as out_s,
        nc.sbuf_tensor([P, IBLK], dt.float16) as comp,
        nc.sbuf_tensor([P, IBLK], dt.float16) as dead,
        nc.semaphore() as dsem,
        nc.semaphore() as vsem,
        nc.Block() as block,
    ):
        xf16_s = xin_s[:, 0:NB16].bitcast(dt.float16)
        xf32_s = xin_s[:, NB16:NB16 + NB32].bitcast(dt.float32)
        tmr_s = xf16_s[:, 0:IBLK]
        rke_s = xf16_s[:, IBLK:2 * IBLK]
        uj_s = xf32_s[:, 0 * CCH:1 * CCH]
        rj_s = xf32_s[:, 1 * CCH:2 * CCH]
        hij_s = xf32_s[:, 2 * CCH:3 * CCH]
        loj_s = xf32_s[:, 3 * CCH:4 * CCH]
        cs_s = out_s[:, 0 * CCH:1 * CCH]
        cc_s = out_s[:, 1 * CCH:2 * CCH]
        ta_s = out_s[:, 2 * CCH:3 * CCH]
        tb_s = out_s[:, 3 * CCH:4 * CCH]

        @block.gpsimd
        def _(g):
            g.dma_start(xin_s[:], xin[:]).then_inc(dsem, 16)
            g.wait_ge(vsem, 1)
            g.dma_start(out[:], out_s[:]).then_inc(dsem, 16)

        @block.vector
        def _(v):
            v.wait_ge(dsem, 16)
            last = None
            for jc in range(CCH):
                col = slice(jc, jc + 1)
                v.tensor_scalar(
                    comp[:], tmr_s, uj_s[:, col], None, Alu.is_lt,
                    op1=Alu.add,  # with accum_out, op1 is the reduce op
                    accum_out=cs_s[:, col],
                )
                v.scalar_tensor_tensor(
                    dead[:], rke_s, rj_s[:, col], comp[:],
                    op0=Alu.is_gt, op1=Alu.mult, accum_out=cc_s[:, col],
                )
                v.scalar_tensor_tensor(
                    dead[:], rke_s, hij_s[:, col], comp[:],
                    op0=Alu.is_le, op1=Alu.mult, accum_out=ta_s[:, col],
                )
                last = v.scalar_tensor_tensor(
                    dead[:], rke_s, loj_s[:, col], comp[:],
                    op0=Alu.is_lt, op1=Alu.mult, accum_out=tb_s[:, col],
                )
            last.then_inc(vsem, 1)

    return nc


def _prep_inputs(event_indicator, event_time, estimate):
    d = np.asarray(event_indicator).reshape(-1).astype(bool)
    t = np.asarray(event_time, dtype=np.float32).reshape(-1)
    r = np.asarray(estimate, dtype=np.float32).reshape(-1)
    n = t.shape[0]
    assert n == N

    # t dense ranks.  exp is strictly increasing and injective on the
    # reference's 0.05-grid log-times, so ranking the raw times preserves
    # both the order and the equality structure of t = exp(event_time).
    tv = np.unique(t)
    trk = np.searchsorted(tv, t).astype(np.float32)
    assert len(tv) + 1.0 < 2040.0, "t ranks must stay fp16-exact"
    u = (trk + np.float32(0.5) * (~d).astype(np.float32)).astype(np.float16)
    tm = np.where(d, trk, np.float32(32768.0)).astype(np.float16)

    # r dense ranks -> monotone fp16 embedding (normal range, no denormals).
    rv = np.unique(r)
    m = len(rv)
    assert m + 1024 < 31744, "r rank embedding must stay in normal fp16 range"
    emb = (np.arange(m, dtype=np.uint16) + np.uint16(1024)).view(np.float16)
    rrk = np.searchsorted(rv, r)

    # Tie windows: for each unique value k, the set of m with
    # |fl(rv[m] - rv[k])| <= 1e-8f is contiguous; two-pointer sweep using
    # the same IEEE f32 subtract as the reference's rdiff.
    thr = np.float32(1e-8)
    lo = np.zeros(m, dtype=np.int64)
    hi = np.zeros(m, dtype=np.int64)
    p = 0
    for k in range(m):
        while np.abs(rv[k] - rv[p]) > thr:
            p += 1
        lo[k] = p
    p = m - 1
    for k in range(m - 1, -1, -1):
        while np.abs(rv[k] - rv[p]) > thr:
            p -= 1
        hi[k] = p

    r_e = emb[rrk]
    lo_e = emb[lo[rrk]]
    hi_e = emb[hi[rrk]]

    def jscalar(x):
        # j = jc*128 + p  ->  element [p, jc] of a [128, 64] tile
        # (f32: compare-op scalar operands must be float32; fp16->f32 is exact)
        return np.ascontiguousarray(x.reshape(CCH, P).T.astype(np.float32))

    uj = jscalar(u)
    rj = jscalar(r_e)
    hij = jscalar(hi_e)
    loj = jscalar(lo_e)

    xf32 = np.ascontiguousarray(np.concatenate([uj, rj, hij, loj], axis=1))
    b32 = xf32.view(np.uint8).reshape(P, -1)
    in_maps = []
    for c in range(NCORES):
        blk = slice(c * IBLK, (c + 1) * IBLK)
        row16 = np.concatenate([tm[blk], r_e[blk]])
        b16 = np.ascontiguousarray(
            np.broadcast_to(row16[None, :], (P, 2 * IBLK))).view(np.uint8)
        in_maps.append({
            "xin": np.ascontiguousarray(np.concatenate([b16, b32], axis=1)),
        })
    return in_maps


def _finish(results):
    tot = np.float64(0.0)
    conc = np.float64(0.0)
    tie_a = np.float64(0.0)
    tie_b = np.float64(0.0)
    for res in results:
        o = res["out"].astype(np.float64)
        tot += o[:, 0 * CCH:1 * CCH].sum()
        conc += o[:, 1 * CCH:2 * CCH].sum()
        tie_a += o[:, 2 * CCH:3 * CCH].sum()
        tie_b += o[:, 3 * CCH:4 * CCH].sum()
    tied = tie_a - tie_b
    disc = tot - conc - tied
    loss = (disc + 0.5 * tied) / (disc + conc + tied + 1e-7)
    return np.asarray(1.0 - loss, dtype=np.float32)


def kernel(event_indicator, event_time, estimate):
    from concourse.bass_utils import run_bass_kernel_spmd

    in_maps = _prep_inputs(event_indicator, event_time, estimate)
    if "nc" not in _CACHE:
        _CACHE["nc"] = _build_nc()
    nc = _CACHE["nc"]
    out = run_bass_kernel_spmd(nc, in_maps, core_ids=list(range(NCORES)))
    return _finish(out.results)
